# revision 36
# baseline (speedup 1.0000x reference)
"""Trainium2 Bass kernel for CirculantMultiHeadAttention.

Strategy
--------
Host side: the block-circulant weights (4,4,512) are materialized into dense
(2048,2048) matrices, because on TRN2 a dense matmul on the PE array beats any
FFT formulation by a wide margin (the FFT's pointwise stage would swamp the
vector engines).  Work is sharded over the 8 NeuronCores as (batch b in {0,1})
x (head-group g in {0..3}, 4 heads each): core c = 4*b + g.  Each core
computes q/k/v projections for its 4 heads, RoPE, causal attention, and a
*partial* output projection (contracting only its own 512 context features).
The host sums the 4 partials per batch.

Device side (per core, one Bass program, SPMD over 8 cores):
  - all matmul operands in bf16 (PSUM accumulation stays fp32).  bf16 runs at
    1 PE cycle/row at ANY moving width (fp32r drops to 4 cycles/row below
    256-wide, which hits the narrow causal-diagonal tiles), and it halves
    every DMA byte so q/k/v stay SBUF-resident with no DRAM bounce.
  - projections: out = W_slice @ x as lhsT.T @ rhs, contraction (model dim,
    16 k-tiles of 128) on partitions.  q/k evicted with fused RoPE straight
    into resident SBUF tiles q_all/k_all [128, T] per head; v into [t, feat]
    tiles.  Host permutes W rows per head to (even feats, odd feats) so the
    rotation is two block copies + mul/add.
  - DMA queues: x + wv on SP, wq + wo + mask on Pool (gpsimd), cos/sin + wk
    on Activation -- so the v-projection weights never queue behind wk (a
    9.5us PE stall in the fp32r baseline).
  - attention in scores-transposed layout: S_T[k, q] = k_tile.T @ q_chunk,
    P_T = exp(S_T * scale) (ScalarE), causal masking by a precomputed
    triangular strip, PV accumulation ctxT[d, q] += v_tile.T @ P_T, softmax
    denominators via a ones-vector matmul on PE.  No running-max: scores are
    O(6) for this data, exp is safe in fp32.
  - output projection: psum[t, n] += ctxT_tile.T @ woT_tile.
"""

import os
import sys

import numpy as np

for _p in ("/opt/trn_rl_repo", "/root/.axon_site/_ro/trn_rl_repo"):
    if os.path.isdir(_p) and _p not in sys.path:
        sys.path.insert(0, _p)

import ml_dtypes

import concourse.bass as bass
import concourse.tile as tile
from concourse import bacc, bass_isa, mybir
from concourse.bass_utils import run_bass_kernel_spmd

F32 = mybir.dt.float32
AF = mybir.ActivationFunctionType

# Problem geometry (hardcoded per spec).
B, T_FULL, D = 2, 2048, 2048
H, HD = 16, 128
NCORES = 8
HG = 4                    # heads per core
FS = HG * HD              # 512 feature dims per core
P = 128                   # partitions
KT = D // P               # 16 contraction tiles for projections
SCALE = 1.0 / float(np.sqrt(HD))
MASKW = 896               # triangular mask strip width: 512 + 3*128

# Matmul operand dtype.  bfloat16: 1 cycle/row at any moving width on the PE
# (fp32r needs >=256-wide or pays 4x), half the DMA/SBUF of fp32.  HW/sim
# end-to-end relative error ~1e-3 vs the 2e-2 budget.  CIRC_MM_DT=float32r
# restores the TF32-like mode.
MM_DT = os.environ.get("CIRC_MM_DT", "bfloat16")


def _mm_dt():
    return getattr(mybir.dt, MM_DT)


def _np_dt():
    return ml_dtypes.bfloat16 if MM_DT == "bfloat16" else np.float32


# ---------------------------------------------------------------------------
# Device program
# ---------------------------------------------------------------------------

def _body(es, tc, io, T):
    nc = tc.nc
    ntc = T // 512            # t-chunks of 512
    nkt = T // P              # 128-wide t/k tiles
    mdt = _mm_dt()

    xT, wqT, wkT, wvT, woT, cos2, sin2, maskR, out = io

    # ---- persistent SBUF tiles ------------------------------------------
    const = es.enter_context(tc.tile_pool(name="const", bufs=1))
    mask_sb = const.tile([P, MASKW], mdt, tag="maskR", name="mask_sb")

    # q/k stay SBUF-resident across phases in [feat, t] layout, one tile per
    # head; v in [t, feat] tiles.  No DRAM bounce.
    qkp = es.enter_context(tc.tile_pool(name="qkall", bufs=HG))
    q_all = [qkp.tile([P, T], mdt, tag="qall", name="q_all") for _ in range(HG)]
    k_all = [qkp.tile([P, T], mdt, tag="kall", name="k_all") for _ in range(HG)]
    vap = es.enter_context(tc.tile_pool(name="vall", bufs=nkt))
    v_all = [None] * nkt

    wop = es.enter_context(tc.tile_pool(name="wo", bufs=HG * 4))
    wo_sb = [[wop.tile([P, 512], mdt, tag="wo", name="wo_sb")
              for _ in range(4)] for _ in range(HG)]
    ctxp = es.enter_context(tc.tile_pool(name="ctx", bufs=HG))
    ctx_sb = [ctxp.tile([P, T], mdt, tag="ctx", name="ctx_sb")
              for _ in range(HG)]

    with (
        tc.tile_pool(name="wq", bufs=1) as wqp,
        tc.tile_pool(name="wk", bufs=1) as wkp,
        tc.tile_pool(name="wv", bufs=1) as wvp,
        tc.tile_pool(name="xt", bufs=16) as xtp,
        tc.tile_pool(name="pev", bufs=2) as evp,
        tc.tile_pool(name="trig", bufs=2) as trigp,
        tc.tile_pool(name="pT", bufs=10) as pTp,
        tc.tile_pool(name="pacc", bufs=2) as paccp,
        tc.tile_pool(name="amisc", bufs=2) as amp,
        tc.tile_pool(name="oev", bufs=10) as oevp,
        tc.tile_pool(name="pps", bufs=4, space="PSUM") as psp,
        tc.tile_pool(name="sps", bufs=2, space="PSUM") as sps,
        tc.tile_pool(name="cps", bufs=2, space="PSUM") as cps,
    ):
        # ---- input DMAs: x + wv on SP, wq + wo + mask on Pool (gpsimd),
        # cos/sin + wk on Activation, so the v-projection weights never
        # queue behind wk and the PE can start on x[0]/wq[0] immediately.
        x_first = [xtp.tile([P, 512], mdt, tag="xt", name="x_sb")
                   for _ in range(KT)]
        for m in range(KT):
            nc.sync.dma_start(out=x_first[m][:],
                              in_=xT[m * P:(m + 1) * P, 0:512])
        wq_sb = [wqp.tile([P, FS], mdt, tag="wq", name="wq_sb", bufs=KT)
                 for _ in range(KT)]
        wk_sb = [wkp.tile([P, FS], mdt, tag="wk", name="wk_sb", bufs=KT)
                 for _ in range(KT)]
        wv_sb = [wvp.tile([P, FS], mdt, tag="wv", name="wv_sb", bufs=KT)
                 for _ in range(KT)]
        for m in range(KT):
            nc.gpsimd.dma_start(out=wq_sb[m][:],
                                in_=wqT[m * P:(m + 1) * P, :])
        trig_sb = [None] * ntc
        cos_sb0 = trigp.tile([P, 512], F32, tag="cos", name="cos_sb")
        nc.scalar.dma_start(out=cos_sb0[:], in_=cos2[:, 0:512])
        sin_sb0 = trigp.tile([P, 512], F32, tag="sin", name="sin_sb")
        nc.scalar.dma_start(out=sin_sb0[:], in_=sin2[:, 0:512])
        trig_sb[0] = (cos_sb0, sin_sb0)
        for m in range(KT):
            nc.scalar.dma_start(out=wk_sb[m][:], in_=wkT[m * P:(m + 1) * P, :])
        for m in range(KT):
            nc.sync.dma_start(out=wv_sb[m][:], in_=wvT[m * P:(m + 1) * P, :])
        for dt_i in range(HG):
            for ncj in range(4):
                nc.gpsimd.dma_start(
                    out=wo_sb[dt_i][ncj][:],
                    in_=woT[dt_i * P:(dt_i + 1) * P,
                            ncj * 512:(ncj + 1) * 512])
        nc.gpsimd.dma_start(out=mask_sb[:], in_=maskR[:, :])

        # ---- emitter builders -------------------------------------------
        def proj_chunk_emitters(tci):
            """12 closures: 8 q/k head-projections (fused RoPE) + 4 v."""
            tsl = slice(tci * 512, (tci + 1) * 512)
            if tci == 0:
                x_sb = x_first
            else:
                x_sb = [xtp.tile([P, 512], mdt, tag="xt", name="x_sb")
                        for _ in range(KT)]

            def prefetch():
                if tci > 0:
                    cos_sb = trigp.tile([P, 512], F32, tag="cos",
                                        name="cos_sb")
                    nc.scalar.dma_start(out=cos_sb[:], in_=cos2[:, tsl])
                    sin_sb = trigp.tile([P, 512], F32, tag="sin",
                                        name="sin_sb")
                    nc.scalar.dma_start(out=sin_sb[:], in_=sin2[:, tsl])
                    trig_sb[tci] = (cos_sb, sin_sb)
                    for m in range(KT):
                        eng = nc.sync if m % 2 == 0 else nc.scalar
                        eng.dma_start(out=x_sb[m][:],
                                      in_=xT[m * P:(m + 1) * P, tsl])

            def qk_em(wsb, dst, h):
                def em():
                    cos_sb, sin_sb = trig_sb[tci]
                    hsl = slice(h * P, (h + 1) * P)
                    ps = psp.tile([P, 512], F32, tag="ps", name="ps")
                    for m in range(KT):
                        nc.tensor.matmul(ps[:], wsb[m][:, hsl], x_sb[m][:],
                                         start=(m == 0), stop=(m == KT - 1))
                    # RoPE: rot = [-odd; even] of ps, fused into eviction
                    rot = evp.tile([P, 512], F32, tag="rot", name="rot")
                    nc.scalar.mul(rot[0:64, :], ps[64:128, :], -1.0)
                    nc.scalar.copy(rot[64:128, :], ps[0:64, :])
                    o = evp.tile([P, 512], F32, tag="o", name="o")
                    nc.vector.tensor_mul(o[:], ps[:], cos_sb[:])
                    nc.vector.tensor_mul(rot[:], rot[:], sin_sb[:])
                    nc.vector.tensor_add(dst[h][:, tsl], o[:], rot[:])
                return em

            def v_em(ts):
                def em():
                    tt = tci * 4 + ts
                    ps = psp.tile([P, FS], F32, tag="ps", name="ps")
                    for m in range(KT):
                        nc.tensor.matmul(ps[:],
                                         x_sb[m][:, ts * P:(ts + 1) * P],
                                         wv_sb[m][:],
                                         start=(m == 0), stop=(m == KT - 1))
                    vt = vap.tile([P, FS], mdt, tag="vall", name="v_all")
                    nc.vector.tensor_copy(vt[:], ps[:])
                    v_all[tt] = vt
                return em

            ems = []
            first = [prefetch]
            for wsb, dst in ((wq_sb, q_all), (wk_sb, k_all)):
                for h in range(HG):
                    e = qk_em(wsb, dst, h)
                    if first:
                        pf = first.pop()
                        ems.append(lambda e=e, pf=pf: (pf(), e()))
                    else:
                        ems.append(e)
            for ts in range(4):
                ems.append(v_em(ts))
            return ems

        def outproj_psum(tt, ncj, final=False, evict_act=False):
            # one psum group of 4 matmuls (~850ns of dep-free PE work) --
            # the filler currency interleaved into the attention, whose exp
            # chain runs ~25% slower on ACT than the PE eats S+PV pairs.
            nsl = slice(ncj * 512, (ncj + 1) * 512)
            pool = sps if (final and ncj % 2) else psp
            ps = pool.tile([P, 512], F32, tag=pool is sps and "sps" or "ps",
                           name="ops")
            for dt_i in range(HG):
                nc.tensor.matmul(ps[:],
                                 ctx_sb[dt_i][:, tt * P:(tt + 1) * P],
                                 wo_sb[dt_i][ncj][:],
                                 start=(dt_i == 0), stop=(dt_i == HG - 1))
            if final:
                # drain latency: evict each psum as two halves on DVE+ACT
                # in parallel, DMAs fanned over SP+Pool
                for hi, (a, b) in enumerate(((0, 256), (256, 512))):
                    o = oevp.tile([P, 256], mdt, tag="of", name="o",
                                  bufs=8)
                    if (ncj + hi) % 2 == 0:
                        nc.vector.tensor_copy(o[:], ps[:, a:b])
                    else:
                        nc.scalar.copy(o[:], ps[:, a:b])
                    eng = nc.sync if hi == 0 else nc.gpsimd
                    eng.dma_start(out=out[tt * P:(tt + 1) * P,
                                          ncj * 512 + a:ncj * 512 + b],
                                  in_=o[:])
            else:
                o = oevp.tile([P, 512], mdt, tag="o", name="o")
                if ncj % 2 == 0 and not evict_act:
                    nc.vector.tensor_copy(o[:], ps[:])
                else:
                    nc.scalar.copy(o[:], ps[:])
                nc.sync.dma_start(out=out[tt * P:(tt + 1) * P, nsl],
                                  in_=o[:])

        def attn_head_emitters(h, qc):
            """nmg+1 closures; micro-group i = S+exp for kt pair i, with
            the masked PV + denominator accumulation pipelined one step
            behind.  PE filler (outproj of tile ftt) is woven in before the
            early S pairs; ftt is shifted back one tile so the h=0 head of
            each chunk fills with a tile whose ctx is long finished."""
            qsl = slice(qc * 512, (qc + 1) * 512)
            nk = 4 * (qc + 1)
            nmg = nk // 2
            hsl = slice(h * P, (h + 1) * P)
            ftt = 4 * (qc - 1) + h - 1
            fillers = list(range(4)) if ftt >= 0 else []
            st = {}

            def tile_slices(kt):
                j = kt - 4 * qc
                c0 = 128 * j if j > 0 else 0
                return slice(qc * 512 + c0, (qc + 1) * 512), slice(c0, 512), c0

            def s_pair(i):
                for kt in (2 * i, 2 * i + 1):
                    lsl, psl, c0 = tile_slices(kt)
                    s_ps = sps.tile([P, 512], F32, tag="sps", name="s_ps")
                    nc.tensor.matmul(s_ps[:, psl],
                                     k_all[h][:, kt * P:(kt + 1) * P],
                                     q_all[h][:, lsl], start=True, stop=True)
                    p_t = pTp.tile([P, 512], mdt, tag="pT", name="p_t")
                    nc.scalar.activation(p_t[:, psl], s_ps[:, psl], AF.Exp,
                                         scale=SCALE)
                    st[kt] = p_t

            def pv_pair(i):
                kts = (2 * i, 2 * i + 1)
                for kt in kts:
                    _, psl, c0 = tile_slices(kt)
                    if kt >= 4 * qc:
                        nc.vector.tensor_mul(st[kt][:, psl], st[kt][:, psl],
                                             mask_sb[:, 384:384 + 512 - c0])
                    nc.tensor.matmul(st["ctx"][:, psl], v_all[kt][:, hsl],
                                     st[kt][:, psl],
                                     start=(kt == 0), stop=(kt == nk - 1))
                # denominator accumulation (off the PE): bf16 pair add at
                # 2x DVE rate where both tiles are full-width, fp32 into
                # the accumulator
                k0, k1 = kts
                full = k1 < 4 * qc + 1   # both tiles full 512 wide
                if full and k0 > 0:
                    t1 = pTp.tile([P, 512], mdt, tag="pT", name="t1")
                    nc.vector.tensor_add(t1[:], st[k0][:], st[k1][:])
                    nc.vector.tensor_add(st["pacc"][:], st["pacc"][:], t1[:])
                else:
                    for kt in kts:
                        _, psl, _ = tile_slices(kt)
                        if kt == 0:
                            nc.vector.tensor_copy(st["pacc"][:], st[kt][:])
                        else:
                            nc.vector.tensor_add(st["pacc"][:, psl],
                                                 st["pacc"][:, psl],
                                                 st[kt][:, psl])
                for kt in kts:
                    del st[kt]

            def em_i(i):
                def em():
                    if i == 0:
                        st["ctx"] = cps.tile([P, 512], F32, tag="cps",
                                             name="ctx_ps")
                        st["pacc"] = paccp.tile([P, 512], F32, tag="pacc",
                                                name="pacc")
                    if i < nmg:
                        if fillers and i < nmg - 1:
                            rem = max(nmg - 1 - i, 1)
                            nf = (len(fillers) + rem - 1) // rem
                            for _ in range(nf):
                                outproj_psum(ftt, fillers.pop(0))
                        s_pair(i)
                    if i > 0:
                        pv_pair(i - 1)
                    if i == nmg:
                        rs_red = amp.tile([P, 512], F32, tag="rs",
                                          name="rs_red")
                        nc.gpsimd.partition_all_reduce(
                            rs_red[:], st["pacc"][:], channels=P,
                            reduce_op=bass_isa.ReduceOp.add)
                        nc.vector.reciprocal(rs_red[:], rs_red[:])
                        nc.vector.tensor_mul(ctx_sb[h][:, qsl],
                                             st["ctx"][:], rs_red[:])
                return em
            return [em_i(i) for i in range(nmg + 1)]

        # ---- schedule ----------------------------------------------------
        # chunks 0-2: pure projections
        for tci in range(3):
            for em in proj_chunk_emitters(tci):
                em()
        # chunk 3 projections interleaved with qc=0 attention (each
        # attention micro-step lands between two ~3.4us projection psum
        # groups, so its exp/mask chain is fully hidden)
        pe_ems = proj_chunk_emitters(3)
        at_ems = [em for h in range(HG) for em in attn_head_emitters(h, 0)]
        n = max(len(pe_ems), len(at_ems))
        for i in range(n):
            if i < len(pe_ems):
                pe_ems[i]()
            if i < len(at_ems):
                at_ems[i]()
        # chunks 1-3 attention with outproj fillers
        for qc in range(1, ntc):
            for h in range(HG):
                for em in attn_head_emitters(h, qc):
                    em()
        # final output projection: tiles 11..15
        for tt in range(4 * (ntc - 1) - 1, 4 * ntc):
            for ncj in range(4):
                outproj_psum(tt, ncj, final=True)


def build_program(T=T_FULL):
    from contextlib import ExitStack

    nc = bacc.Bacc("TRN2", target_bir_lowering=False, debug=False,
                   num_devices=NCORES)
    mdt = _mm_dt()
    xT = nc.dram_tensor("xT", (D, T), mdt, kind="ExternalInput").ap()
    wqT = nc.dram_tensor("wqT", (D, FS), mdt, kind="ExternalInput").ap()
    wkT = nc.dram_tensor("wkT", (D, FS), mdt, kind="ExternalInput").ap()
    wvT = nc.dram_tensor("wvT", (D, FS), mdt, kind="ExternalInput").ap()
    woT = nc.dram_tensor("woT", (FS, D), mdt, kind="ExternalInput").ap()
    cos2 = nc.dram_tensor("cos2", (P, T), F32, kind="ExternalInput").ap()
    sin2 = nc.dram_tensor("sin2", (P, T), F32, kind="ExternalInput").ap()
    maskR = nc.dram_tensor("maskR", (P, MASKW), mdt,
                           kind="ExternalInput").ap()
    out = nc.dram_tensor("out", (T, D), mdt, kind="ExternalOutput").ap()

    io = (xT, wqT, wkT, wvT, woT, cos2, sin2, maskR, out)
    with tile.TileContext(nc) as tc:
        with ExitStack() as es:
            _body(es, tc, io, T)
    nc.compile()
    return nc


# ---------------------------------------------------------------------------
# Host-side data prep
# ---------------------------------------------------------------------------

def dense_from_circulant(w):
    """(qb, pb, bs) generating vectors -> dense (qb*bs, pb*bs) matrix."""
    w = np.asarray(w, dtype=np.float32)
    qb, pb, bs = w.shape
    idx = (np.arange(bs)[:, None] - np.arange(bs)[None, :]) % bs
    blocks = w[:, :, idx]                      # (qb, pb, bs, bs)
    return np.ascontiguousarray(
        blocks.transpose(0, 2, 1, 3).reshape(qb * bs, pb * bs))


_EO_PERM = np.concatenate([np.arange(0, HD, 2), np.arange(1, HD, 2)])


def _perm_rows_even_odd(w_rows):
    """Permute each 128-row head block to (even rows, odd rows)."""
    nh = w_rows.shape[0] // HD
    blocks = w_rows.reshape(nh, HD, -1)[:, _EO_PERM, :]
    return blocks.reshape(w_rows.shape)


def rope_tables(T=T_FULL, theta=10000.0):
    inv = 1.0 / (theta ** (np.arange(0, HD, 2, dtype=np.float32) / HD))
    ang = np.arange(T, dtype=np.float32)[:, None] * inv[None, :]
    cos = np.cos(ang).astype(np.float32).T      # (64, T)
    sin = np.sin(ang).astype(np.float32).T
    cos2 = np.ascontiguousarray(np.concatenate([cos, cos], axis=0))
    sin2 = np.ascontiguousarray(np.concatenate([sin, sin], axis=0))
    return cos2, sin2


def mask_strip():
    kk = np.arange(P)[:, None]
    c = np.arange(MASKW)[None, :]
    return np.ascontiguousarray(((c - 384) >= kk).astype(np.float32))


def make_in_maps(x, w_q, w_k, w_v, w_o, T=T_FULL):
    """Build the 8 per-core input maps from full inputs."""
    ndt = _np_dt()
    x = np.asarray(x, dtype=np.float32)
    Wq = dense_from_circulant(w_q)
    Wk = dense_from_circulant(w_k)
    Wv = dense_from_circulant(w_v)
    Wo = dense_from_circulant(w_o)
    cos2, sin2 = rope_tables(T)
    mstrip = mask_strip()

    xTb = [np.ascontiguousarray(x[b, :T, :].T.astype(ndt)) for b in range(B)]
    in_maps = []
    for c in range(NCORES):
        b, g = divmod(c, NCORES // B)
        fs = slice(FS * g, FS * (g + 1))
        in_maps.append({
            "xT": xTb[b],
            "wqT": np.ascontiguousarray(
                _perm_rows_even_odd(Wq[fs, :]).T.astype(ndt)),
            "wkT": np.ascontiguousarray(
                _perm_rows_even_odd(Wk[fs, :]).T.astype(ndt)),
            "wvT": np.ascontiguousarray(Wv[fs, :].T.astype(ndt)),
            "woT": np.ascontiguousarray(Wo[:, fs].T.astype(ndt)),
            "cos2": cos2,
            "sin2": sin2,
            "maskR": mstrip.astype(ndt),
        })
    return in_maps


_PROGRAM_CACHE = {}


def get_program(T=T_FULL):
    key = (T, MM_DT)
    if key not in _PROGRAM_CACHE:
        _PROGRAM_CACHE[key] = build_program(T)
    return _PROGRAM_CACHE[key]


LAST_EXEC_NS = None


def kernel(x, w_q, w_k, w_v, w_o, mask=None, trace=False):
    """Full inputs in, full output out.  Shards over 8 NeuronCores."""
    global LAST_EXEC_NS
    x = np.asarray(x, dtype=np.float32)
    in_maps = make_in_maps(x, w_q, w_k, w_v, w_o, T_FULL)
    nc = get_program(T_FULL)
    try:
        res = run_bass_kernel_spmd(nc, in_maps, core_ids=list(range(NCORES)),
                                   trace=trace)
    except ModuleNotFoundError:
        # no NTFF profiling hook in this container; run untraced
        res = run_bass_kernel_spmd(nc, in_maps, core_ids=list(range(NCORES)),
                                   trace=False)
    LAST_EXEC_NS = res.exec_time_ns
    gpb = NCORES // B
    out = np.stack([
        sum(np.asarray(res.results[b * gpb + g]["out"], dtype=np.float64)
            for g in range(gpb)).astype(np.float32)
        for b in range(B)
    ])
    return out


# revision 37
# speedup vs baseline: 1.0148x; 1.0148x over previous
"""Trainium2 Bass kernel for CirculantMultiHeadAttention.

Strategy
--------
Host side: the block-circulant weights (4,4,512) are materialized into dense
(2048,2048) matrices, because on TRN2 a dense matmul on the PE array beats any
FFT formulation by a wide margin (the FFT's pointwise stage would swamp the
vector engines).  Work is sharded over the 8 NeuronCores as (batch b in {0,1})
x (head-group g in {0..3}, 4 heads each): core c = 4*b + g.  Each core
computes q/k/v projections for its 4 heads, RoPE, causal attention, and a
*partial* output projection (contracting only its own 512 context features).
The host sums the 4 partials per batch.

Device side (per core, one Bass program, SPMD over 8 cores):
  - all matmul operands in bf16 (PSUM accumulation stays fp32).  bf16 runs at
    1 PE cycle/row at ANY moving width (fp32r drops to 4 cycles/row below
    256-wide, which hits the narrow causal-diagonal tiles), and it halves
    every DMA byte so q/k/v stay SBUF-resident with no DRAM bounce.
  - projections: out = W_slice @ x as lhsT.T @ rhs, contraction (model dim,
    16 k-tiles of 128) on partitions.  q/k evicted with fused RoPE straight
    into resident SBUF tiles q_all/k_all [128, T] per head; v into [t, feat]
    tiles.  Host permutes W rows per head to (even feats, odd feats) so the
    rotation is two block copies + mul/add.
  - DMA queues: x + wv on SP, wq + wo + mask on Pool (gpsimd), cos/sin + wk
    on Activation -- so the v-projection weights never queue behind wk (a
    9.5us PE stall in the fp32r baseline).
  - attention in scores-transposed layout: S_T[k, q] = k_tile.T @ q_chunk,
    P_T = exp(S_T * scale) (ScalarE), causal masking by a precomputed
    triangular strip, PV accumulation ctxT[d, q] += v_tile.T @ P_T, softmax
    denominators via a ones-vector matmul on PE.  No running-max: scores are
    O(6) for this data, exp is safe in fp32.
  - output projection: psum[t, n] += ctxT_tile.T @ woT_tile.
"""

import os
import sys

import numpy as np

for _p in ("/opt/trn_rl_repo", "/root/.axon_site/_ro/trn_rl_repo"):
    if os.path.isdir(_p) and _p not in sys.path:
        sys.path.insert(0, _p)

import ml_dtypes

import concourse.bass as bass
import concourse.tile as tile
from concourse import bacc, bass_isa, mybir
from concourse.bass_utils import run_bass_kernel_spmd

F32 = mybir.dt.float32
AF = mybir.ActivationFunctionType

# Problem geometry (hardcoded per spec).
B, T_FULL, D = 2, 2048, 2048
H, HD = 16, 128
NCORES = 8
HG = 4                    # heads per core
FS = HG * HD              # 512 feature dims per core
P = 128                   # partitions
KT = D // P               # 16 contraction tiles for projections
SCALE = 1.0 / float(np.sqrt(HD))
MASKW = 896               # triangular mask strip width: 512 + 3*128

# Matmul operand dtype.  bfloat16: 1 cycle/row at any moving width on the PE
# (fp32r needs >=256-wide or pays 4x), half the DMA/SBUF of fp32.  HW/sim
# end-to-end relative error ~1e-3 vs the 2e-2 budget.  CIRC_MM_DT=float32r
# restores the TF32-like mode.
MM_DT = os.environ.get("CIRC_MM_DT", "bfloat16")


def _mm_dt():
    return getattr(mybir.dt, MM_DT)


def _np_dt():
    return ml_dtypes.bfloat16 if MM_DT == "bfloat16" else np.float32


# ---------------------------------------------------------------------------
# Device program
# ---------------------------------------------------------------------------

def _body(es, tc, io, T):
    nc = tc.nc
    ntc = T // 512            # t-chunks of 512
    nkt = T // P              # 128-wide t/k tiles
    mdt = _mm_dt()

    xT, wqT, wkT, wvT, woT, cos2, sin2, maskR, out = io

    # ---- persistent SBUF tiles ------------------------------------------
    const = es.enter_context(tc.tile_pool(name="const", bufs=1))
    mask_sb = const.tile([P, MASKW], mdt, tag="maskR", name="mask_sb")

    # q/k stay SBUF-resident across phases in [feat, t] layout, one tile per
    # head; v in [t, feat] tiles.  No DRAM bounce.
    qkp = es.enter_context(tc.tile_pool(name="qkall", bufs=HG))
    q_all = [qkp.tile([P, T], mdt, tag="qall", name="q_all") for _ in range(HG)]
    k_all = [qkp.tile([P, T], mdt, tag="kall", name="k_all") for _ in range(HG)]
    vap = es.enter_context(tc.tile_pool(name="vall", bufs=nkt))
    v_all = [None] * nkt

    wop = es.enter_context(tc.tile_pool(name="wo", bufs=HG * 4))
    wo_sb = [[wop.tile([P, 512], mdt, tag="wo", name="wo_sb")
              for _ in range(4)] for _ in range(HG)]
    ctxp = es.enter_context(tc.tile_pool(name="ctx", bufs=HG))
    ctx_sb = [ctxp.tile([P, T], mdt, tag="ctx", name="ctx_sb")
              for _ in range(HG)]

    with (
        tc.tile_pool(name="wq", bufs=1) as wqp,
        tc.tile_pool(name="wk", bufs=1) as wkp,
        tc.tile_pool(name="wv", bufs=1) as wvp,
        tc.tile_pool(name="xt", bufs=16) as xtp,
        tc.tile_pool(name="pev", bufs=2) as evp,
        tc.tile_pool(name="trig", bufs=2) as trigp,
        tc.tile_pool(name="pT", bufs=10) as pTp,
        tc.tile_pool(name="pacc", bufs=2) as paccp,
        tc.tile_pool(name="amisc", bufs=2) as amp,
        tc.tile_pool(name="oev", bufs=10) as oevp,
        tc.tile_pool(name="pps", bufs=4, space="PSUM") as psp,
        tc.tile_pool(name="sps", bufs=2, space="PSUM") as sps,
        tc.tile_pool(name="cps", bufs=2, space="PSUM") as cps,
    ):
        # ---- input DMAs: x + wv on SP, wq + wo + mask on Pool (gpsimd),
        # cos/sin + wk on Activation, so the v-projection weights never
        # queue behind wk and the PE can start on x[0]/wq[0] immediately.
        x_first = [xtp.tile([P, 512], mdt, tag="xt", name="x_sb")
                   for _ in range(KT)]
        for m in range(KT):
            nc.sync.dma_start(out=x_first[m][:],
                              in_=xT[m * P:(m + 1) * P, 0:512])
        wq_sb = [wqp.tile([P, FS], mdt, tag="wq", name="wq_sb", bufs=KT)
                 for _ in range(KT)]
        wk_sb = [wkp.tile([P, FS], mdt, tag="wk", name="wk_sb", bufs=KT)
                 for _ in range(KT)]
        wv_sb = [wvp.tile([P, FS], mdt, tag="wv", name="wv_sb", bufs=KT)
                 for _ in range(KT)]
        for m in range(KT):
            nc.gpsimd.dma_start(out=wq_sb[m][:],
                                in_=wqT[m * P:(m + 1) * P, :])
        trig_sb = [None] * ntc
        cos_sb0 = trigp.tile([P, 512], F32, tag="cos", name="cos_sb")
        nc.scalar.dma_start(out=cos_sb0[:], in_=cos2[:, 0:512])
        sin_sb0 = trigp.tile([P, 512], F32, tag="sin", name="sin_sb")
        nc.scalar.dma_start(out=sin_sb0[:], in_=sin2[:, 0:512])
        trig_sb[0] = (cos_sb0, sin_sb0)
        for m in range(KT):
            nc.scalar.dma_start(out=wk_sb[m][:], in_=wkT[m * P:(m + 1) * P, :])
        for m in range(KT):
            nc.sync.dma_start(out=wv_sb[m][:], in_=wvT[m * P:(m + 1) * P, :])
        for dt_i in range(HG):
            for ncj in range(4):
                nc.gpsimd.dma_start(
                    out=wo_sb[dt_i][ncj][:],
                    in_=woT[dt_i * P:(dt_i + 1) * P,
                            ncj * 512:(ncj + 1) * 512])
        nc.gpsimd.dma_start(out=mask_sb[:], in_=maskR[:, :])

        # ---- emitter builders -------------------------------------------
        def proj_chunk_emitters(tci):
            """12 closures: 8 q/k head-projections (fused RoPE) + 4 v."""
            tsl = slice(tci * 512, (tci + 1) * 512)
            if tci == 0:
                x_sb = x_first
            else:
                x_sb = [xtp.tile([P, 512], mdt, tag="xt", name="x_sb")
                        for _ in range(KT)]

            def prefetch():
                if tci > 0:
                    cos_sb = trigp.tile([P, 512], F32, tag="cos",
                                        name="cos_sb")
                    nc.scalar.dma_start(out=cos_sb[:], in_=cos2[:, tsl])
                    sin_sb = trigp.tile([P, 512], F32, tag="sin",
                                        name="sin_sb")
                    nc.scalar.dma_start(out=sin_sb[:], in_=sin2[:, tsl])
                    trig_sb[tci] = (cos_sb, sin_sb)
                    for m in range(KT):
                        eng = nc.sync if m % 2 == 0 else nc.scalar
                        eng.dma_start(out=x_sb[m][:],
                                      in_=xT[m * P:(m + 1) * P, tsl])

            def qk_em(wsb, dst, h):
                def em():
                    cos_sb, sin_sb = trig_sb[tci]
                    hsl = slice(h * P, (h + 1) * P)
                    ps = psp.tile([P, 512], F32, tag="ps", name="ps")
                    for m in range(KT):
                        nc.tensor.matmul(ps[:], wsb[m][:, hsl], x_sb[m][:],
                                         start=(m == 0), stop=(m == KT - 1))
                    # RoPE: rot = [-odd; even] of ps, fused into eviction
                    rot = evp.tile([P, 512], F32, tag="rot", name="rot")
                    nc.scalar.mul(rot[0:64, :], ps[64:128, :], -1.0)
                    nc.scalar.copy(rot[64:128, :], ps[0:64, :])
                    o = evp.tile([P, 512], F32, tag="o", name="o")
                    nc.vector.tensor_mul(o[:], ps[:], cos_sb[:])
                    nc.vector.tensor_mul(rot[:], rot[:], sin_sb[:])
                    nc.vector.tensor_add(dst[h][:, tsl], o[:], rot[:])
                return em

            def v_em(ts):
                def em():
                    tt = tci * 4 + ts
                    ps = psp.tile([P, FS], F32, tag="ps", name="ps")
                    for m in range(KT):
                        nc.tensor.matmul(ps[:],
                                         x_sb[m][:, ts * P:(ts + 1) * P],
                                         wv_sb[m][:],
                                         start=(m == 0), stop=(m == KT - 1))
                    vt = vap.tile([P, FS], mdt, tag="vall", name="v_all")
                    nc.vector.tensor_copy(vt[:], ps[:])
                    v_all[tt] = vt
                return em

            ems = []
            first = [prefetch]
            for wsb, dst in ((wq_sb, q_all), (wk_sb, k_all)):
                for h in range(HG):
                    e = qk_em(wsb, dst, h)
                    if first:
                        pf = first.pop()
                        ems.append(lambda e=e, pf=pf: (pf(), e()))
                    else:
                        ems.append(e)
            for ts in range(4):
                ems.append(v_em(ts))
            return ems

        def outproj_psum(tt, ncj, final=False, evict_act=False):
            # one psum group of 4 matmuls (~850ns of dep-free PE work) --
            # the filler currency interleaved into the attention, whose exp
            # chain runs ~25% slower on ACT than the PE eats S+PV pairs.
            nsl = slice(ncj * 512, (ncj + 1) * 512)
            pool = sps if (final and ncj % 2) else psp
            ps = pool.tile([P, 512], F32, tag=pool is sps and "sps" or "ps",
                           name="ops")
            for dt_i in range(HG):
                nc.tensor.matmul(ps[:],
                                 ctx_sb[dt_i][:, tt * P:(tt + 1) * P],
                                 wo_sb[dt_i][ncj][:],
                                 start=(dt_i == 0), stop=(dt_i == HG - 1))
            o = oevp.tile([P, 512], mdt, tag="o", name="o")
            if ncj % 2 == 0 and not evict_act:
                nc.vector.tensor_copy(o[:], ps[:])
            else:
                nc.scalar.copy(o[:], ps[:])
            eng = nc.gpsimd if (final and ncj % 2) else nc.sync
            eng.dma_start(out=out[tt * P:(tt + 1) * P, nsl], in_=o[:])

        def attn_head_emitters(h, qc):
            """nmg+1 closures; micro-group i = S+exp for kt pair i, with
            the masked PV + denominator accumulation pipelined one step
            behind.  PE filler (outproj of tile ftt) is woven in before the
            early S pairs; ftt is shifted back one tile so the h=0 head of
            each chunk fills with a tile whose ctx is long finished."""
            qsl = slice(qc * 512, (qc + 1) * 512)
            nk = 4 * (qc + 1)
            nmg = nk // 2
            hsl = slice(h * P, (h + 1) * P)
            ftt = 4 * (qc - 1) + h - 1
            fillers = list(range(4)) if ftt >= 0 else []
            st = {}

            def tile_slices(kt):
                j = kt - 4 * qc
                c0 = 128 * j if j > 0 else 0
                return slice(qc * 512 + c0, (qc + 1) * 512), slice(c0, 512), c0

            def s_pair(i):
                for kt in (2 * i, 2 * i + 1):
                    lsl, psl, c0 = tile_slices(kt)
                    s_ps = sps.tile([P, 512], F32, tag="sps", name="s_ps")
                    nc.tensor.matmul(s_ps[:, psl],
                                     k_all[h][:, kt * P:(kt + 1) * P],
                                     q_all[h][:, lsl], start=True, stop=True)
                    p_t = pTp.tile([P, 512], mdt, tag="pT", name="p_t")
                    nc.scalar.activation(p_t[:, psl], s_ps[:, psl], AF.Exp,
                                         scale=SCALE)
                    st[kt] = p_t

            def pv_pair(i):
                kts = (2 * i, 2 * i + 1)
                for kt in kts:
                    _, psl, c0 = tile_slices(kt)
                    if kt >= 4 * qc:
                        nc.vector.tensor_mul(st[kt][:, psl], st[kt][:, psl],
                                             mask_sb[:, 384:384 + 512 - c0])
                    nc.tensor.matmul(st["ctx"][:, psl], v_all[kt][:, hsl],
                                     st[kt][:, psl],
                                     start=(kt == 0), stop=(kt == nk - 1))
                # denominator accumulation (off the PE): bf16 pair add at
                # 2x DVE rate where both tiles are full-width, fp32 into
                # the accumulator
                k0, k1 = kts
                full = k1 < 4 * qc + 1   # both tiles full 512 wide
                if full and k0 > 0:
                    t1 = pTp.tile([P, 512], mdt, tag="pT", name="t1")
                    nc.vector.tensor_add(t1[:], st[k0][:], st[k1][:])
                    nc.vector.tensor_add(st["pacc"][:], st["pacc"][:], t1[:])
                else:
                    for kt in kts:
                        _, psl, _ = tile_slices(kt)
                        if kt == 0:
                            nc.vector.tensor_copy(st["pacc"][:], st[kt][:])
                        else:
                            nc.vector.tensor_add(st["pacc"][:, psl],
                                                 st["pacc"][:, psl],
                                                 st[kt][:, psl])
                for kt in kts:
                    del st[kt]

            def em_i(i):
                def em():
                    if i == 0:
                        st["ctx"] = cps.tile([P, 512], F32, tag="cps",
                                             name="ctx_ps")
                        st["pacc"] = paccp.tile([P, 512], F32, tag="pacc",
                                                name="pacc")
                    if i < nmg:
                        if fillers and i < nmg - 1:
                            rem = max(nmg - 1 - i, 1)
                            nf = (len(fillers) + rem - 1) // rem
                            for _ in range(nf):
                                outproj_psum(ftt, fillers.pop(0))
                        s_pair(i)
                    if i > 0:
                        pv_pair(i - 1)
                    if i == nmg:
                        rs_red = amp.tile([P, 512], F32, tag="rs",
                                          name="rs_red")
                        nc.gpsimd.partition_all_reduce(
                            rs_red[:], st["pacc"][:], channels=P,
                            reduce_op=bass_isa.ReduceOp.add)
                        nc.vector.reciprocal(rs_red[:], rs_red[:])
                        nc.vector.tensor_mul(ctx_sb[h][:, qsl],
                                             st["ctx"][:], rs_red[:])
                return em
            return [em_i(i) for i in range(nmg + 1)]

        # ---- schedule ----------------------------------------------------
        # chunks 0-2: pure projections
        for tci in range(3):
            for em in proj_chunk_emitters(tci):
                em()
        # chunk 3 projections interleaved with qc=0 attention (each
        # attention micro-step lands between two ~3.4us projection psum
        # groups, so its exp/mask chain is fully hidden)
        pe_ems = proj_chunk_emitters(3)
        at_ems = [em for h in range(HG) for em in attn_head_emitters(h, 0)]
        n = max(len(pe_ems), len(at_ems))
        for i in range(n):
            if i < len(pe_ems):
                pe_ems[i]()
            if i < len(at_ems):
                at_ems[i]()
        # chunks 1-3 attention with outproj fillers
        for qc in range(1, ntc):
            for h in range(HG):
                for em in attn_head_emitters(h, qc):
                    em()
        # final output projection: tiles 11..15
        for tt in range(4 * (ntc - 1) - 1, 4 * ntc):
            for ncj in range(4):
                outproj_psum(tt, ncj, final=True)


def build_program(T=T_FULL):
    from contextlib import ExitStack

    nc = bacc.Bacc("TRN2", target_bir_lowering=False, debug=False,
                   num_devices=NCORES)
    mdt = _mm_dt()
    xT = nc.dram_tensor("xT", (D, T), mdt, kind="ExternalInput").ap()
    wqT = nc.dram_tensor("wqT", (D, FS), mdt, kind="ExternalInput").ap()
    wkT = nc.dram_tensor("wkT", (D, FS), mdt, kind="ExternalInput").ap()
    wvT = nc.dram_tensor("wvT", (D, FS), mdt, kind="ExternalInput").ap()
    woT = nc.dram_tensor("woT", (FS, D), mdt, kind="ExternalInput").ap()
    cos2 = nc.dram_tensor("cos2", (P, T), F32, kind="ExternalInput").ap()
    sin2 = nc.dram_tensor("sin2", (P, T), F32, kind="ExternalInput").ap()
    maskR = nc.dram_tensor("maskR", (P, MASKW), mdt,
                           kind="ExternalInput").ap()
    out = nc.dram_tensor("out", (T, D), mdt, kind="ExternalOutput").ap()

    io = (xT, wqT, wkT, wvT, woT, cos2, sin2, maskR, out)
    with tile.TileContext(nc) as tc:
        with ExitStack() as es:
            _body(es, tc, io, T)
    nc.compile()
    return nc


# ---------------------------------------------------------------------------
# Host-side data prep
# ---------------------------------------------------------------------------

def dense_from_circulant(w):
    """(qb, pb, bs) generating vectors -> dense (qb*bs, pb*bs) matrix."""
    w = np.asarray(w, dtype=np.float32)
    qb, pb, bs = w.shape
    idx = (np.arange(bs)[:, None] - np.arange(bs)[None, :]) % bs
    blocks = w[:, :, idx]                      # (qb, pb, bs, bs)
    return np.ascontiguousarray(
        blocks.transpose(0, 2, 1, 3).reshape(qb * bs, pb * bs))


_EO_PERM = np.concatenate([np.arange(0, HD, 2), np.arange(1, HD, 2)])


def _perm_rows_even_odd(w_rows):
    """Permute each 128-row head block to (even rows, odd rows)."""
    nh = w_rows.shape[0] // HD
    blocks = w_rows.reshape(nh, HD, -1)[:, _EO_PERM, :]
    return blocks.reshape(w_rows.shape)


def rope_tables(T=T_FULL, theta=10000.0):
    inv = 1.0 / (theta ** (np.arange(0, HD, 2, dtype=np.float32) / HD))
    ang = np.arange(T, dtype=np.float32)[:, None] * inv[None, :]
    cos = np.cos(ang).astype(np.float32).T      # (64, T)
    sin = np.sin(ang).astype(np.float32).T
    cos2 = np.ascontiguousarray(np.concatenate([cos, cos], axis=0))
    sin2 = np.ascontiguousarray(np.concatenate([sin, sin], axis=0))
    return cos2, sin2


def mask_strip():
    kk = np.arange(P)[:, None]
    c = np.arange(MASKW)[None, :]
    return np.ascontiguousarray(((c - 384) >= kk).astype(np.float32))


def make_in_maps(x, w_q, w_k, w_v, w_o, T=T_FULL):
    """Build the 8 per-core input maps from full inputs."""
    ndt = _np_dt()
    x = np.asarray(x, dtype=np.float32)
    Wq = dense_from_circulant(w_q)
    Wk = dense_from_circulant(w_k)
    Wv = dense_from_circulant(w_v)
    Wo = dense_from_circulant(w_o)
    cos2, sin2 = rope_tables(T)
    mstrip = mask_strip()

    xTb = [np.ascontiguousarray(x[b, :T, :].T.astype(ndt)) for b in range(B)]
    in_maps = []
    for c in range(NCORES):
        b, g = divmod(c, NCORES // B)
        fs = slice(FS * g, FS * (g + 1))
        in_maps.append({
            "xT": xTb[b],
            "wqT": np.ascontiguousarray(
                _perm_rows_even_odd(Wq[fs, :]).T.astype(ndt)),
            "wkT": np.ascontiguousarray(
                _perm_rows_even_odd(Wk[fs, :]).T.astype(ndt)),
            "wvT": np.ascontiguousarray(Wv[fs, :].T.astype(ndt)),
            "woT": np.ascontiguousarray(Wo[:, fs].T.astype(ndt)),
            "cos2": cos2,
            "sin2": sin2,
            "maskR": mstrip.astype(ndt),
        })
    return in_maps


_PROGRAM_CACHE = {}


def get_program(T=T_FULL):
    key = (T, MM_DT)
    if key not in _PROGRAM_CACHE:
        _PROGRAM_CACHE[key] = build_program(T)
    return _PROGRAM_CACHE[key]


LAST_EXEC_NS = None


def kernel(x, w_q, w_k, w_v, w_o, mask=None, trace=False):
    """Full inputs in, full output out.  Shards over 8 NeuronCores."""
    global LAST_EXEC_NS
    x = np.asarray(x, dtype=np.float32)
    in_maps = make_in_maps(x, w_q, w_k, w_v, w_o, T_FULL)
    nc = get_program(T_FULL)
    try:
        res = run_bass_kernel_spmd(nc, in_maps, core_ids=list(range(NCORES)),
                                   trace=trace)
    except ModuleNotFoundError:
        # no NTFF profiling hook in this container; run untraced
        res = run_bass_kernel_spmd(nc, in_maps, core_ids=list(range(NCORES)),
                                   trace=False)
    LAST_EXEC_NS = res.exec_time_ns
    gpb = NCORES // B
    out = np.stack([
        sum(np.asarray(res.results[b * gpb + g]["out"], dtype=np.float64)
            for g in range(gpb)).astype(np.float32)
        for b in range(B)
    ])
    return out
